# revision 10
# baseline (speedup 1.0000x reference)
"""Mistral sparse-MoE (B=4,S=2048,H=1024,F=4096,E=8,top-2) on 8 trn2 cores.

Expert-parallel sharding: core e holds expert e's gate/up/down weights.
The host computes the (tiny) router + top-2 dispatch and uses it to shard:
each core receives exactly the tokens routed to its expert (gathered,
transposed, zero-padded to a common capacity C), the expert weights in
K-major partition-blocked bf16 layout, and the per-token combine weights.
The device kernel computes the full expert FFN
  y = (silu(x@gW^T) * (x@uW^T)) @ dW^T * w
for its tokens; the host scatter-adds the 8 partial outputs back into the
[T, H] result (pure unshard of the expert-parallel partial sums).

Schedule: the F dimension is processed in NQ=8 blocks of FB=512 columns.
For each block, gate/up/down weight slices are DMAed once (total weight
traffic per iteration = one copy of all weights, ~24 MB), the token range
is swept in ~416-wide chunks (PSUM-bank sized), and the down-projection
partial products are accumulated into an SBUF f32 accumulator [H, C].
Down-proj for chunk i is issued after gate/up for chunk i+1 so the PE
never waits on the silu/mul tail of the current chunk. After the last
block the accumulator is scaled by the per-token combine weight and
written out as bf16.

DRAM layouts are partition-major ([128, k, free]) so every DMA is a
contiguous-run-per-partition access pattern. An optional hardware repeat
loop (tc.For_i) re-runs the full kernel body for benchmarking.
"""

import numpy as np
import ml_dtypes
from contextlib import ExitStack

B, S, H, F, E, TOPK = 4, 2048, 1024, 4096, 8, 2
T = B * S
P = 128
KH = H // P        # 8  contraction chunks for gate/up
KF = F // P        # 32 contraction chunks for down
HM = H // P        # 8  output row tiles
FB = 512           # f-columns per weight block
NQ = F // FB       # 8  weight blocks
KFB = FB // P      # 4  down-contraction chunks per block

_BF16 = ml_dtypes.bfloat16


NN_TARGET = 352    # token-chunk width target (<=512; 352 gives an even
                   # chunk count at C~2080 so every chunk gets a pair)


def _chunks(C, target=None):
    """Split C token columns into near-even 16-aligned chunks of <=target."""
    target = target or NN_TARGET
    n_ch = max(1, (C + target - 1) // target)
    per = -(-C // n_ch)
    per = ((per + 15) // 16) * 16
    out = []
    n0 = 0
    while n0 < C:
        nn = min(per, C - n0)
        out.append((n0, nn))
        n0 += nn
    return out


def _build_program(C, repeat=1):
    import concourse.tile as tile
    from concourse import bacc, mybir

    bf16 = mybir.dt.bfloat16
    f32 = mybir.dt.float32

    nc = bacc.Bacc("TRN2", target_bir_lowering=False, debug=False, num_devices=E)

    xT = nc.dram_tensor("xT", [P, KH, C], bf16, kind="ExternalInput").ap()
    gw = nc.dram_tensor("gw", [P, KH, F], bf16, kind="ExternalInput").ap()
    uw = nc.dram_tensor("uw", [P, KH, F], bf16, kind="ExternalInput").ap()
    dw = nc.dram_tensor("dw", [P, KF, H], bf16, kind="ExternalInput").ap()
    wr = nc.dram_tensor("wr", [P, C], f32, kind="ExternalInput").ap()
    yT = nc.dram_tensor("yT", [P, HM, C], bf16, kind="ExternalOutput").ap()

    chunks = _chunks(C)

    with tile.TileContext(nc) as tc, ExitStack() as ctx:
        xp = ctx.enter_context(tc.tile_pool(name="xp", bufs=1))
        wp = ctx.enter_context(tc.tile_pool(name="wp", bufs=1))
        yap = ctx.enter_context(tc.tile_pool(name="yap", bufs=1))
        gwp = ctx.enter_context(tc.tile_pool(name="gwp", bufs=2))
        uwp = ctx.enter_context(tc.tile_pool(name="uwp", bufs=2))
        dwp = ctx.enter_context(tc.tile_pool(name="dwp", bufs=2))
        sgp = ctx.enter_context(tc.tile_pool(name="sgp", bufs=4))
        hp = ctx.enter_context(tc.tile_pool(name="hp", bufs=2))
        ytp = ctx.enter_context(tc.tile_pool(name="ytp", bufs=2))
        pg = ctx.enter_context(tc.tile_pool(name="pg", bufs=1, space="PSUM"))
        pu = ctx.enter_context(tc.tile_pool(name="pu", bufs=1, space="PSUM"))
        py = ctx.enter_context(tc.tile_pool(name="py", bufs=2, space="PSUM"))

        # token chunks processed in pairs: the two members' accumulation
        # groups are interleaved instruction-by-instruction so each weight
        # tile loaded into the PE array serves two matmuls back-to-back
        # (the second load of an identical stationary is cheap on HW).
        pairs = [tuple(chunks[i:i + 2]) for i in range(0, len(chunks), 2)]

        def body():
            wt = wp.tile([P, C], f32)
            nc.sync.dma_start(out=wt[:], in_=wr[:, :])
            xt = xp.tile([P, KH, C], bf16)
            for (n0, nn) in chunks:
                nc.sync.dma_start(out=xt[:, :, n0:n0 + nn],
                                  in_=xT[:, :, n0:n0 + nn])
            ya = yap.tile([P, HM, C], f32)

            def emit_down(qd, dt, pair, h_pair):
                yts = []
                if qd == NQ - 1:
                    yts = [ytp.tile([P, HM, nn], bf16, tag=f"yt{m}", name=f"yt{m}")
                           for m, (n0, nn) in enumerate(pair)]
                for hm in range(HM):
                    psys = [py.tile([P, nn], f32, tag=f"psy{m}", name=f"psy{m}")
                            for m, (n0, nn) in enumerate(pair)]
                    for kf in range(KFB):
                        for m, (n0, nn) in enumerate(pair):
                            nc.tensor.matmul(
                                psys[m][:], dt[:, kf, hm * P:(hm + 1) * P],
                                h_pair[m][kf][:],
                                start=(kf == 0), stop=(kf == KFB - 1),
                                skip_group_check=True)
                    for m, (n0, nn) in enumerate(pair):
                        ya_s = ya[:, hm, n0:n0 + nn]
                        if qd == 0:
                            nc.scalar.copy(ya_s, psys[m][:])
                        else:
                            nc.vector.tensor_add(ya_s, ya_s, psys[m][:])
                            if qd == NQ - 1:
                                nc.vector.tensor_mul(
                                    yts[m][:, hm, :], ya_s, wt[:, n0:n0 + nn])
                if qd == NQ - 1:
                    for m, (n0, nn) in enumerate(pair):
                        nc.sync.dma_start(out=yT[:, :, n0:n0 + nn],
                                          in_=yts[m][:])

            pend = None
            for q in range(NQ):
                f0 = q * FB
                gt = gwp.tile([P, KH, FB], bf16)
                nc.sync.dma_start(out=gt[:], in_=gw[:, :, f0:f0 + FB])
                ut = uwp.tile([P, KH, FB], bf16)
                nc.sync.dma_start(out=ut[:], in_=uw[:, :, f0:f0 + FB])
                dt = dwp.tile([P, KFB, H], bf16)
                nc.sync.dma_start(out=dt[:], in_=dw[:, q * KFB:(q + 1) * KFB, :])

                for pi, pair in enumerate(pairs):
                    h_pair = [[] for _ in pair]
                    for fm in range(KFB):
                        psgs = [pg.tile([P, nn], f32, tag=f"psg{m}", name=f"psg{m}")
                                for m, (n0, nn) in enumerate(pair)]
                        psus = [pu.tile([P, nn], f32, tag=f"psu{m}", name=f"psu{m}")
                                for m, (n0, nn) in enumerate(pair)]
                        for k in range(KH):
                            for m, (n0, nn) in enumerate(pair):
                                nc.tensor.matmul(
                                    psgs[m][:], gt[:, k, fm * P:(fm + 1) * P],
                                    xt[:, k, n0:n0 + nn],
                                    start=(k == 0), stop=(k == KH - 1),
                                    skip_group_check=True)
                        for k in range(KH):
                            for m, (n0, nn) in enumerate(pair):
                                nc.tensor.matmul(
                                    psus[m][:], ut[:, k, fm * P:(fm + 1) * P],
                                    xt[:, k, n0:n0 + nn],
                                    start=(k == 0), stop=(k == KH - 1),
                                    skip_group_check=True)
                        for m, (n0, nn) in enumerate(pair):
                            sg = sgp.tile([P, nn], bf16, tag=f"sg{m}")
                            nc.scalar.activation(
                                sg[:], psgs[m][:],
                                mybir.ActivationFunctionType.Silu)
                            ht = hp.tile([P, nn], bf16,
                                         tag=f"h{pi % 2}_{m}_{fm}")
                            nc.vector.tensor_mul(ht[:], sg[:], psus[m][:])
                            h_pair[m].append(ht)
                    if pend is not None:
                        emit_down(*pend)
                    pend = (q, dt, pair, h_pair)
            emit_down(*pend)

        if repeat == 1:
            body()
        else:
            with tc.For_i(0, repeat):
                body()

    nc.finalize()
    return nc


def _route(x, router_w):
    # top-2 routing in f64 (exactly ties-stable vs the fp32 reference for
    # any non-degenerate logits)
    logits = x.astype(np.float64) @ router_w.T.astype(np.float64)
    rows = np.arange(T)
    i1 = np.argmax(logits, axis=1)
    v1 = logits[rows, i1]
    masked = logits.copy()
    masked[rows, i1] = -np.inf
    i2 = np.argmax(masked, axis=1)
    v2 = masked[rows, i2]
    e2 = np.exp(v2 - v1)
    w1 = 1.0 / (1.0 + e2)
    w2 = e2 / (1.0 + e2)
    return i1, i2, w1.astype(np.float32), w2.astype(np.float32)


def _pmajor(a, kdim):
    """[K*128, N] -> [128, K, N] partition-major contiguous."""
    k, n = a.shape
    return np.ascontiguousarray(
        a.reshape(kdim, P, n).transpose(1, 0, 2))


def _prepare(hidden_states, router_w, gate_w, up_w, down_w):
    """Route tokens and build per-core input maps. Returns (in_maps, idxs, C)."""
    x = np.asarray(hidden_states, dtype=np.float32).reshape(T, H)
    router_w = np.asarray(router_w, dtype=np.float32)

    i1, i2, w1, w2 = _route(x, router_w)

    idxs, wts = [], []
    for e in range(E):
        m1 = i1 == e
        m2 = i2 == e
        idx = np.nonzero(m1 | m2)[0]
        w = np.where(m1[idx], w1[idx], w2[idx])
        idxs.append(idx)
        wts.append(w)

    max_ne = max(len(i) for i in idxs)
    C = max(512, ((max_ne + 15) // 16) * 16)

    x_bf = x.astype(_BF16)
    in_maps = []
    for e in range(E):
        idx, w = idxs[e], wts[e]
        n_e = len(idx)
        xTe = np.zeros((H, C), dtype=_BF16)
        xTe[:, :n_e] = x_bf[idx].T
        wre = np.zeros((P, C), dtype=np.float32)
        wre[:, :n_e] = w[None, :]
        in_maps.append({
            "xT": _pmajor(xTe, KH),
            "gw": _pmajor(np.asarray(gate_w)[e].T.astype(_BF16), KH),
            "uw": _pmajor(np.asarray(up_w)[e].T.astype(_BF16), KH),
            "dw": _pmajor(np.asarray(down_w)[e].T.astype(_BF16), KF),
            "wr": wre,
        })
    return in_maps, idxs, C


def _combine(results, idxs, C):
    """Scatter-add the 8 per-expert partial outputs into the full [B,S,H]."""
    out = np.zeros((T, H), dtype=np.float32)
    for e in range(E):
        idx = idxs[e]
        # yT dram is [128, HM, C] partition-major -> [H, C]
        yTe = results[e]["yT"].transpose(1, 0, 2).reshape(H, C)
        out[idx] += yTe[:, :len(idx)].astype(np.float32).T
    return out.reshape(B, S, H)


def kernel(hidden_states, router_w, gate_w, up_w, down_w):
    from concourse.bass_utils import run_bass_kernel_spmd

    in_maps, idxs, C = _prepare(hidden_states, router_w, gate_w, up_w, down_w)
    nc = _build_program(C)
    results = run_bass_kernel_spmd(nc, in_maps, list(range(E))).results
    return _combine(results, idxs, C)
